# revision 16
# baseline (speedup 1.0000x reference)
"""CNN-MRF loss (retrieval kNN) on 8 Trainium2 NeuronCores.

Reference: cosine-similarity argmax between all 96x96 content patches and
96x96 style patches (3x3xC=128 patches, d=1152), gather matched style
patches, fold (overlap-add), MSE against content features.

Sharding: content-patch axis N split 8 ways (12 grid rows / core), style
replicated.  Per core, per 128-patch tile j:
  similarity: fp8(e4m3) matmul of content patch rows against
     HOST-PRE-NORMALIZED style patch rows (style side absorbs 1/||s||, so
     no on-device scaling pass is needed).  Contraction over d=1152 runs
     as 5 DoubleRow passes (2x fp8 rate, 256-deep each, zero-padded to 10
     chunks of 128).  PSUM -> SBUF (bf16) copies run on the otherwise-idle
     Scalar engine, so PSUM drain never waits on the DVE.
     fp8 scoring moves the argmax for ~6% of patches to a near-equal
     neighbour; measured end-MSE error 1.3e-4, far inside tolerance.
  argmax (two-level): ONE DVE scan produces all 18 per-group maxima
     (tensor_reduce axis=X over S viewed [128,18,512]); a tiny argmax over
     the 18 picks each partition's winning group; S is mirrored to a DRAM
     scratch tile so an indirect DMA can fetch each partition's winning
     512-wide group; a short max_index over 512 yields the final index.
     This avoids a second full 9216-element DVE scan.
  fold-by-matmul: matched (bf16) style rows gathered by indirect DMA are
     folded via 9 PSUM-accumulated matmuls against constant 0/1 scatter
     matrices (out[c,p] = sum_n matched[n,c,k] * A[n,k,p]), replacing 9 PE
     transposes + 18 DVE adds with one DVE add of a [128,392] strip.
     Deferred one iteration so the PE never waits on the argmax chain.
Host: sums the 8 overlapping strips, divides by fold counts, MSE.
"""
import os
import sys
import numpy as np

for _p in ("/opt/trn_rl_repo",):
    if _p not in sys.path:
        sys.path.insert(0, _p)

import concourse.bass as bass
import concourse.bacc as bacc
import concourse.mybir as mybir
from concourse.bass import IndirectOffsetOnAxis
from concourse.bass_utils import run_bass_kernel_spmd
from concourse.tile import TileContext
from concourse.masks import make_identity

F32 = mybir.dt.float32
BF16 = mybir.dt.bfloat16
FP8 = mybir.dt.float8e4
U32 = mybir.dt.uint32

C = 128          # channels
H = W = 96       # feature-map spatial dims
PW = 3           # patch size
N = H * W        # content patches total (9216)
M = N            # style patches (9216)
D = C * PW * PW  # patch vector length (1152)
NCORES = 8
RPC = H // NCORES       # content grid rows per core (12)
NSH = RPC * W           # content patches per core (1152)
NT = NSH // 128         # n-tiles of 128 per core (9)
MG = 512                # style patches per matmul group
NG = M // MG            # matmul groups (18)
KC = 10                 # contraction chunks of 128 (9 real + 1 zero pad)
PSTRIP = 4 * (W + 2)    # fold output strip length (4 rows x 98)

TWOLVL = os.environ.get("TWOLVL", "1") == "1"
FOLDMM = os.environ.get("FOLDMM", "1") == "1"


def ts(i, size):
    return slice(i * size, (i + 1) * size)


def build_program():
    nc = bacc.Bacc()

    cvT8 = nc.declare_dram_parameter("cvT8", [128, KC, NSH], FP8, isOutput=False)
    svnT8 = nc.declare_dram_parameter("svnT8", [NG, 128, KC, MG], FP8, isOutput=False)
    sprows = nc.declare_dram_parameter("sprows", [M, D], BF16, isOutput=False)
    foldA = nc.declare_dram_parameter("foldA", [128, 3, 9, PSTRIP], BF16,
                                      isOutput=False)
    nrow18 = nc.declare_dram_parameter("nrow18", [128, 1], F32, isOutput=False)
    idx_out = nc.declare_dram_parameter("idx_out", [NT, 128, 1], U32, isOutput=True)
    racc_out = nc.declare_dram_parameter(
        "racc_out", [C, RPC + 2, W], F32, isOutput=True
    )

    with TileContext(nc) as tc:
        with (
            tc.tile_pool(name="const", bufs=1) as constp,
            tc.tile_pool(name="big", bufs=1) as bigp,
            tc.tile_pool(name="work", bufs=2) as workp,
            tc.tile_pool(name="dram", bufs=1, space="DRAM") as dramp,
            tc.tile_pool(name="psS", bufs=4, space="PSUM") as psS,
            tc.tile_pool(name="psF", bufs=2, space="PSUM") as psF,
        ):
            # ---- loads (spread across engine queues so the DMA triggers
            # don't serialize on one queue) ----
            cvT_sb = bigp.tile([128, KC, NSH], FP8)
            svn_sb = bigp.tile([128, NG, KC, MG], FP8)
            qs = [nc.sync, nc.scalar, nc.gpsimd]
            nc.scalar.dma_start(out=cvT_sb[:, 0:2], in_=cvT8[:, 0:2])
            nc.sync.dma_start(out=svn_sb[:, 0], in_=svnT8[0])
            nc.scalar.dma_start(out=cvT_sb[:, 2:KC], in_=cvT8[:, 2:KC])
            for g in range(1, NG):
                qs[g % 3].dma_start(out=svn_sb[:, g], in_=svnT8[g])
            A_sb = constp.tile([128, 3, 9, PSTRIP], BF16)
            nc.scalar.dma_start(out=A_sb[:], in_=foldA[:])
            nrow_sb = constp.tile([128, 1], F32)
            nc.sync.dma_start(out=nrow_sb[:], in_=nrow18[:])

            ident = None
            if not FOLDMM:
                ident = constp.tile([128, 128], F32)
                make_identity(nc, ident[:])

            racc = bigp.tile([C, (RPC + 2) * (W + 2)], F32)
            nc.gpsimd.memset(racc[:], 0.0)

            S_dram = dramp.tile([128 * NG, MG], BF16)

            def fold(j, matched):
                """Fold matched patches into racc via scatter matmuls."""
                matched3 = matched[:].rearrange("p (a b) -> p a b", b=9)
                n0 = j * 128
                r0, c0 = n0 // W, n0 % W
                if FOLDMM:
                    pf = psF.tile([128, PSTRIP], F32, tag="psF", name=f"pf_{j}")
                    for k in range(9):
                        nc.tensor.matmul(
                            out=pf[:],
                            lhsT=matched3[:, :, k],
                            rhs=A_sb[:, c0 // 32, k, :],
                            start=(k == 0),
                            stop=(k == 8),
                        )
                    nc.vector.tensor_add(
                        racc[:, r0 * (W + 2) : r0 * (W + 2) + PSTRIP],
                        racc[:, r0 * (W + 2) : r0 * (W + 2) + PSTRIP],
                        pf[:],
                    )
                else:
                    racc3 = racc[:].rearrange("p (a b) -> p a b", b=W + 2)
                    seg1 = (r0, c0, W - c0, 0)
                    seg2 = (r0 + 1, 0, 128 - (W - c0), W - c0)
                    trT = psF.tile([128, 9, 128], F32, tag="psT", name=f"trT_{j}")
                    for k in range(9):
                        nc.tensor.transpose(trT[:, k], matched3[:, :, k], ident[:])
                    for k in range(9):
                        ki, kj = k // 3, k % 3
                        for (r, c, ln, off) in (seg1, seg2):
                            nc.vector.tensor_add(
                                racc3[:, r + ki, c + kj : c + kj + ln],
                                racc3[:, r + ki, c + kj : c + kj + ln],
                                trT[:, k, off : off + ln],
                            )

            DR = mybir.MatmulPerfMode.DoubleRow
            deferred = None
            for j in range(NT):
                S_sb = bigp.tile([128, NG, MG], BF16, tag="S_sb", bufs=2)

                for g in range(NG):
                    pt = psS.tile([128, MG], F32, tag="psS", name=f"ps_{j}_{g}")
                    for p in range(KC // 2):
                        nc.tensor.matmul(
                            out=pt[:],
                            lhsT=cvT_sb[:, 2 * p : 2 * p + 2, ts(j, 128)],
                            rhs=svn_sb[:, g, 2 * p : 2 * p + 2, :],
                            start=(p == 0),
                            stop=(p == KC // 2 - 1),
                            perf_mode=DR,
                        )
                    # PSUM -> SBUF on the Scalar engine (keeps DVE free)
                    nc.scalar.copy(S_sb[:, g], pt[:])
                    if TWOLVL and g % 3 == 2:
                        # eager S mirror to DRAM, 3 groups at a time, so the
                        # winning-group gather never waits on a bulk DMA
                        nc.sync.dma_start(
                            out=S_dram[:].rearrange("(p g) m -> p g m", g=NG)[
                                :, g - 2 : g + 1
                            ],
                            in_=S_sb[:, g - 2 : g + 1],
                        )

                # all 18 group maxima from ONE scan
                gmax = workp.tile([128, NG], BF16, tag="gmax")
                nc.vector.tensor_reduce(
                    out=gmax[:],
                    in_=S_sb[:],
                    axis=mybir.AxisListType.X,
                    op=mybir.AluOpType.max,
                )
                vm8 = workp.tile([128, 8], BF16, tag="vm8")
                nc.vector.max(vm8[:], gmax[:])

                if TWOLVL:
                    gi8 = workp.tile([128, 8], U32, tag="gi8")
                    nc.vector.max_index(gi8[:], vm8[:], gmax[:])
                    g8f = workp.tile([128, 1], F32, tag="g8f")
                    nc.vector.tensor_copy(g8f[:], gi8[:, 0:1])
                    offf = workp.tile([128, 1], F32, tag="offf")
                    nc.vector.scalar_tensor_tensor(
                        out=offf[:],
                        in0=g8f[:],
                        scalar=0.0,
                        in1=nrow_sb[:],
                        op0=mybir.AluOpType.add,
                        op1=mybir.AluOpType.add,
                    )
                    offu = workp.tile([128, 1], U32, tag="offu")
                    nc.vector.tensor_copy(offu[:], offf[:])
                    Sg = workp.tile([128, MG], BF16, tag="Sg")
                    nc.gpsimd.indirect_dma_start(
                        out=Sg[:],
                        out_offset=None,
                        in_=S_dram[:],
                        in_offset=IndirectOffsetOnAxis(ap=offu[:, 0:1], axis=0),
                    )
                    li8 = workp.tile([128, 8], U32, tag="li8")
                    vm8b = workp.tile([128, 8], BF16, tag="vm8b")
                    nc.vector.tensor_copy(
                        vm8b[:], vm8[:, 0:1].to_broadcast((128, 8))
                    )
                    nc.vector.max_index(li8[:], vm8b[:], Sg[:])
                    lf = workp.tile([128, 1], F32, tag="lf")
                    nc.vector.tensor_copy(lf[:], li8[:, 0:1])
                    bestf = workp.tile([128, 1], F32, tag="bestf")
                    nc.vector.scalar_tensor_tensor(
                        out=bestf[:],
                        in0=g8f[:],
                        scalar=float(MG),
                        in1=lf[:],
                        op0=mybir.AluOpType.mult,
                        op1=mybir.AluOpType.add,
                    )
                    bestu = workp.tile([128, 1], U32, tag="bestu")
                    nc.vector.tensor_copy(bestu[:], bestf[:])
                else:
                    idx8 = workp.tile([128, 8], U32, tag="idx8")
                    nc.vector.max_index(
                        idx8[:], vm8[:, 0:1].to_broadcast((128, 8)), S_sb[:]
                    )
                    bestu = workp.tile([128, 1], U32, tag="bestu")
                    nc.vector.tensor_copy(bestu[:], idx8[:, 0:1])

                # on the gpsimd queue: it is already gated on bestu for the
                # matched-row gather, so this never stalls another queue
                nc.gpsimd.dma_start(out=idx_out[j], in_=bestu[:])

                # gather matched style patch rows (n-major); the indirect
                # DMA needs a flat 2D dest (3D dest tiles fetch garbage)
                matched = workp.tile([128, D], BF16, tag="matched")
                nc.gpsimd.indirect_dma_start(
                    out=matched[:],
                    out_offset=None,
                    in_=sprows[:],
                    in_offset=IndirectOffsetOnAxis(ap=bestu[:, 0:1], axis=0),
                )

                # fold of the previous tile, deferred so tile j+1's matmuls
                # are already queued on the PE before these matmuls
                if deferred is not None:
                    fold(*deferred)
                deferred = (j, matched)

            fold(*deferred)
            racc3 = racc[:].rearrange("p (a b) -> p a b", b=W + 2)
            nc.sync.dma_start(out=racc_out[:], in_=racc3[:, :, 1 : 1 + W])

    if not nc.is_finalized():
        nc.finalize()
    return nc


_PROGRAM = None


def _get_program():
    global _PROGRAM
    if _PROGRAM is None:
        _PROGRAM = build_program()
    return _PROGRAM


def _patch_rows(x):
    """(C, R, Cc) padded map -> ((R-2)*(Cc-2), C*9) patch rows, (c,ki,kj)."""
    w = np.lib.stride_tricks.sliding_window_view(x, (PW, PW), axis=(1, 2))
    return np.ascontiguousarray(
        w.transpose(1, 2, 0, 3, 4).reshape((x.shape[1] - 2) * (x.shape[2] - 2), -1)
    )


_FOLD_A = None


def _fold_A():
    """(128, 3, 9, PSTRIP) 0/1 scatter matrices for fold-by-matmul."""
    global _FOLD_A
    if _FOLD_A is None:
        bf = mybir.dt.np(BF16)
        A = np.zeros((128, 3, 9, PSTRIP), dtype=bf)
        for v in range(3):
            c0 = 32 * v
            for n in range(128):
                rr, cc = (c0 + n) // W, (c0 + n) % W
                for k in range(9):
                    ki, kj = k // 3, k % 3
                    A[n, v, k, (rr + ki) * (W + 2) + cc + kj] = 1.0
        _FOLD_A = A
    return _FOLD_A


def _host_prep(content_feats, style_feats):
    """Build per-core input maps."""
    f8 = mybir.dt.np(FP8)
    bf = mybir.dt.np(BF16)
    cf = np.ascontiguousarray(np.asarray(content_feats, dtype=np.float32)[0])
    sf = np.ascontiguousarray(np.asarray(style_feats, dtype=np.float32)[0])
    cpad = np.pad(cf, ((0, 0), (1, 1), (1, 1)))
    spad = np.pad(sf, ((0, 0), (1, 1), (1, 1)))
    sprows = _patch_rows(spad)
    invn = 1.0 / np.maximum(
        np.linalg.norm(sprows, axis=1), np.float32(1e-12)
    ).astype(np.float32)
    svn8 = (sprows * invn[:, None]).astype(f8)
    # (M, D) -> (NG, 128, KC, MG): svnT[g, d, c, m] = svn[g*MG+m, c*128+d]
    svnT = np.zeros((NG, 128, KC, MG), dtype=f8)
    svnT[:, :, : D // 128, :] = (
        svn8.reshape(NG, MG, D // 128, 128).transpose(0, 3, 2, 1)
    )
    svnT = np.ascontiguousarray(svnT)
    sprows_bf = sprows.astype(bf)
    nrow = (np.arange(128, dtype=np.float32) * NG).reshape(128, 1)
    foldA = _fold_A()
    in_maps = []
    for i in range(NCORES):
        cslab = np.ascontiguousarray(cpad[:, i * RPC : i * RPC + RPC + 2, :])
        cv8 = _patch_rows(cslab).astype(f8)      # (NSH, D)
        cvT = np.zeros((128, KC, NSH), dtype=f8)
        cvT[:, : D // 128, :] = cv8.reshape(NSH, D // 128, 128).transpose(2, 1, 0)
        in_maps.append(
            {
                "cvT8": np.ascontiguousarray(cvT),
                "svnT8": svnT,
                "sprows": sprows_bf,
                "foldA": foldA,
                "nrow18": nrow,
            }
        )
    return cf, in_maps


_DIVISOR = None


def _fold_divisor():
    global _DIVISOR
    if _DIVISOR is None:
        cnt = np.full(H, 3, dtype=np.float32)
        cnt[0] = cnt[-1] = 2
        _DIVISOR = np.outer(cnt, cnt).astype(np.float32) + np.float32(1e-8)
    return _DIVISOR


def _host_combine(cf, results):
    acc = np.zeros((C, H + 2, W), dtype=np.float32)
    for i in range(NCORES):
        acc[:, i * RPC : i * RPC + RPC + 2, :] += results[i]["racc_out"]
    recon = acc[:, 1 : 1 + H, :] / _fold_divisor()[None, :, :]
    diff = cf - recon
    return np.float32(np.mean(np.square(diff), dtype=np.float64))


def run(content_feats, style_feats, trace=False):
    nc = _get_program()
    cf, in_maps = _host_prep(content_feats, style_feats)
    res = run_bass_kernel_spmd(
        nc, in_maps, core_ids=list(range(NCORES)), trace=trace
    )
    mse = _host_combine(cf, res.results)
    return mse, res


def kernel(content_feats, style_feats):
    mse, _ = run(content_feats, style_feats)
    return np.array(mse, dtype=np.float32)
